# revision 26
# baseline (speedup 1.0000x reference)
"""Trainium2 Bass kernel for LoFTR-style linear attention (nn_AttentionLayer).

Data-parallel over B=1024 across 8 NeuronCores (128 batches/core, no
collectives).  All heavy compute in bf16 on the TensorEngine with fp32 PSUM
accumulation.

Key tricks:
  - Host-side transpose: pass xg^T (768, 16384) so projections need no
    on-chip transpose (contraction dim = input features on partitions).
  - phi(x) = elu(x)+1 == max(exp(min(x,0)), 1+x): computed with 2 ACT passes
    (Relu(-z), Exp(-u)) + 1 fused DVE scalar_tensor_tensor (max(z+1, e)).
  - V augmented with a ones column so KV' = phi(K)^T [V|1] yields both KV and
    Ksum in one matmul; U = phi(Q) @ KV' yields both the unnormalized output
    and the normalizer denominator in one matmul.
  - Q kept feature-on-partition (bias via ACT per-partition operand);
    K/V token-on-partition (bias folded into the matmul as a K=1 ones-row).
  - Attention einsums packed 2 heads per matmul via PE row/col tile_position.
"""

import numpy as np
import ml_dtypes

NCORES = 8
B, L, HID, GUID, H, D = 1024, 128, 512, 256, 8, 64
KIN = HID + GUID          # 768
BSH = B // NCORES         # 128 batches per core
TOK = BSH * L             # 16384 tokens per core
BLK = 512                 # tokens per block (4 batches)
EPS = 1e-6

_CACHE = {}


def _build(nblocks):
    from contextlib import ExitStack
    import concourse.bass as bass
    import concourse.mybir as mybir
    import concourse.tile as tile
    from concourse import bacc

    f32 = mybir.dt.float32
    bf16 = mybir.dt.bfloat16
    AF = mybir.ActivationFunctionType
    OP = mybir.AluOpType

    ntok = nblocks * BLK

    nc = bacc.Bacc("TRN2", target_bir_lowering=False, debug=False,
                   num_devices=NCORES)

    xgT = nc.dram_tensor("xgT", (KIN, ntok), bf16, kind="ExternalInput").ap()
    wq_d = nc.dram_tensor("wq", (128, 6, 512), bf16, kind="ExternalInput").ap()
    wk_d = nc.dram_tensor("wk", (128, 6, 512), bf16, kind="ExternalInput").ap()
    wv_d = nc.dram_tensor("wv", (128, 4, 512), bf16, kind="ExternalInput").ap()
    wk7_d = nc.dram_tensor("wk7", (128, 512), bf16, kind="ExternalInput").ap()
    bvb_d = nc.dram_tensor("bvb", (128, 8, 64), bf16, kind="ExternalInput").ap()
    qb_d = nc.dram_tensor("qb", (128, 4, 2), f32, kind="ExternalInput").ap()
    out_d = nc.dram_tensor("out", (ntok, 512), f32, kind="ExternalOutput").ap()

    with tile.TileContext(nc) as tc, ExitStack() as ctx:
        consts = ctx.enter_context(tc.tile_pool(name="consts", bufs=1))
        xg_pool = ctx.enter_context(tc.tile_pool(name="xg", bufs=3))
        qphi_pool = ctx.enter_context(tc.tile_pool(name="qphi", bufs=2))
        tmp_pool = ctx.enter_context(tc.tile_pool(name="tmp", bufs=4))
        kphi_pool = ctx.enter_context(tc.tile_pool(name="kphi", bufs=2))
        vp_pool = ctx.enter_context(tc.tile_pool(name="vp", bufs=2))
        kv_pool = ctx.enter_context(tc.tile_pool(name="kv", bufs=2))
        out_pool = ctx.enter_context(tc.tile_pool(name="outp", bufs=4))
        rcp_pool = ctx.enter_context(tc.tile_pool(name="rcp", bufs=4))
        psq_pool = ctx.enter_context(tc.tile_pool(name="psq", bufs=2, space="PSUM"))
        pskv_pool = ctx.enter_context(tc.tile_pool(name="pskv", bufs=3, space="PSUM"))
        psatt_pool = ctx.enter_context(tc.tile_pool(name="psatt", bufs=1, space="PSUM"))
        psu_pool = ctx.enter_context(tc.tile_pool(name="psu", bufs=2, space="PSUM"))

        wq_t = consts.tile([128, 6, 512], bf16)
        wk_t = consts.tile([128, 6, 512], bf16)
        wv_t = consts.tile([128, 4, 512], bf16)
        wk7_t = consts.tile([128, 512], bf16)
        bvb_t = consts.tile([128, 8, 64], bf16)
        qb_t = consts.tile([128, 4, 2], f32)
        pad_t = consts.tile([128, 128], bf16)
        # block-0 input first so the first matmuls can start ASAP; weight
        # loads split per k-slice across DMA queues, Q weights first
        xg_first = xg_pool.tile([128, 6, 512], bf16, tag="xg")
        xgv0 = xgT[:, 0:BLK].rearrange("(ko p) n -> p ko n", p=128)
        for k in range(6):
            nc.sync.dma_start(xg_first[:, k, :], xgv0[:, k, :])
            nc.sync.dma_start(wq_t[:, k, :], wq_d[:, k, :])
        for k in range(6):
            nc.sync.dma_start(wk_t[:, k, :], wk_d[:, k, :])
            if k < 4:
                nc.sync.dma_start(wv_t[:, k, :], wv_d[:, k, :])
        nc.sync.dma_start(wk7_t[:], wk7_d[:])
        nc.sync.dma_start(bvb_t[:], bvb_d[:])
        nc.sync.dma_start(qb_t[:], qb_d[:])
        # pad_t.T @ wk7 adds the bk bias row to every token: row 0 is ones,
        # rows 1..127 zero; wk7 row 0 holds bk.
        nc.vector.memset(pad_t[:], 0.0)
        nc.vector.memset(pad_t[0:1, :], 1.0)

        # KV' staging tiles with statically-zeroed dual halves (workaround:
        # matmuls with lhsT/rhs at SBUF base partition 64 crash, so U matmuls
        # run full K=128 against rhs whose other-dual rows are zero).
        # Two copies each for manual double-buffering across batches.
        kvE = [consts.tile([128, 4, 65], bf16, name=f"kvE{i}") for i in range(2)]
        kvO = [consts.tile([128, 4, 65], bf16, name=f"kvO{i}") for i in range(2)]
        for i in range(2):
            nc.vector.memset(kvE[i][64:128, :, :], 0.0)
            nc.vector.memset(kvO[i][0:64, :, :], 0.0)

        for j in range(nblocks):
            if j == 0:
                xg_t = xg_first
            else:
                xg_t = xg_pool.tile([128, 6, 512], bf16, tag="xg")
                xgv = xgT[:, j * BLK:(j + 1) * BLK].rearrange(
                    "(ko p) n -> p ko n", p=128)
                for k in range(6):
                    nc.sync.dma_start(xg_t[:, k, :], xgv[:, k, :])

            # ---- Q projection (feature-on-partition) + phi ----
            qphi_t = qphi_pool.tile([128, 4, 512], bf16, tag="qphi")
            for m in range(4):
                ps = psq_pool.tile([128, 512], f32, tag="psq")
                for k in range(6):
                    nc.tensor.matmul(
                        ps[:],
                        wq_t[:, k, m * 128:(m + 1) * 128],
                        xg_t[:, k, :],
                        start=(k == 0), stop=(k == 5),
                    )
                u = tmp_pool.tile([128, 512], f32, tag="tmp")
                nc.scalar.activation(u[:], ps[:], AF.Relu,
                                     bias=qb_t[:, m, 0:1], scale=-1.0)
                e = tmp_pool.tile([128, 512], f32, tag="tmp")
                nc.scalar.activation(e[:], u[:], AF.Exp, scale=-1.0)
                nc.vector.scalar_tensor_tensor(
                    qphi_t[:, m, :], ps[:], qb_t[:, m, 1:2], e[:],
                    OP.add, OP.max,
                )

            for bi in range(4):
                b = j * 4 + bi
                bs = slice(bi * 128, (bi + 1) * 128)

                # ---- K projection (token-on-partition), bias via ones-row ----
                ps_k = pskv_pool.tile([128, 512], f32, tag="pskv")
                for k in range(6):
                    nc.tensor.matmul(ps_k[:], xg_t[:, k, bs], wk_t[:, k, :],
                                     start=(k == 0), stop=False)
                nc.tensor.matmul(ps_k[:], pad_t[:], wk7_t[:],
                                 start=False, stop=True)

                # ---- V projection (bias folded into the V' copy below) ----
                ps_v = pskv_pool.tile([128, 512], f32, tag="pskv")
                for k in range(4):
                    nc.tensor.matmul(ps_v[:], xg_t[:, k, bs], wv_t[:, k, :],
                                     start=(k == 0), stop=(k == 3))

                # ---- phi(K) ----
                u = tmp_pool.tile([128, 512], f32, tag="tmp")
                nc.scalar.activation(u[:], ps_k[:], AF.Relu, scale=-1.0)
                e = tmp_pool.tile([128, 512], f32, tag="tmp")
                nc.scalar.activation(e[:], u[:], AF.Exp, scale=-1.0)
                kphi_t = kphi_pool.tile([128, 512], bf16, tag="kphi")
                nc.vector.scalar_tensor_tensor(
                    kphi_t[:], ps_k[:], 1.0, e[:], OP.add, OP.max)

                # ---- V' = [V + bv | 1] per head ----
                vp_t = vp_pool.tile([128, 8, 65], bf16, tag="vp")
                nc.vector.memset(vp_t[:, :, 64:65], 1.0)
                nc.vector.tensor_tensor(
                    vp_t[:, :, 0:64],
                    ps_v[:].rearrange("p (h d) -> p h d", d=64),
                    bvb_t[:], OP.add)

                # ---- KV' = phi(K)^T @ V' : 2 heads per pair via col tiling ----
                ps_kv_full = psatt_pool.tile([128, 512], f32, tag="psatt",
                                             name="ps_kv")
                ps_kv = ps_kv_full[:, :260]
                for p in range(4):
                    nc.tensor.matmul(
                        ps_kv[0:64, p * 65:(p + 1) * 65],
                        kphi_t[:, p * 128:p * 128 + 64],
                        vp_t[:, 2 * p, :],
                        start=True, stop=True, tile_position=(0, 0))
                    nc.tensor.matmul(
                        ps_kv[64:128, p * 65:(p + 1) * 65],
                        kphi_t[:, p * 128 + 64:(p + 1) * 128],
                        vp_t[:, 2 * p + 1, :],
                        start=True, stop=True, tile_position=(0, 64))
                kvE_t = kvE[b % 2]
                kvO_t = kvO[b % 2]
                # split across ACT and DVE so the two copies run in parallel
                nc.scalar.copy(
                    kvE_t[0:64, :, :],
                    ps_kv[0:64, :].rearrange("p (c j) -> p c j", j=65))
                nc.vector.tensor_copy(
                    kvO_t[64:128, :, :],
                    ps_kv[64:128, :].rearrange("p (c j) -> p c j", j=65))

                # ---- U = phi(Q) @ KV' : full K=128 against zero-padded KV ----
                out_t = out_pool.tile([128, 512], f32, tag="outp")
                for half in range(2):
                    ps_u_full = psu_pool.tile([128, 512], f32, tag="psu",
                                              name="ps_u")
                    ps_u = ps_u_full[:, :260]
                    for pp in range(2):
                        p = half * 2 + pp
                        nc.tensor.matmul(
                            ps_u[:, pp * 130:pp * 130 + 65],
                            qphi_t[:, p, bs],
                            kvE_t[:, p, :],
                            start=True, stop=True)
                        nc.tensor.matmul(
                            ps_u[:, pp * 130 + 65:pp * 130 + 130],
                            qphi_t[:, p, bs],
                            kvO_t[:, p, :],
                            start=True, stop=True)
                    ps_u3 = ps_u[:].rearrange("p (c j) -> p c j", j=65)
                    d_t = rcp_pool.tile([128, 4], f32, tag="rcp")
                    nc.vector.tensor_scalar_add(d_t[:], ps_u3[:, :, 64], EPS)
                    r_t = rcp_pool.tile([128, 4], f32, tag="rcp")
                    nc.vector.reciprocal(r_t[:], d_t[:])
                    nc.vector.tensor_tensor(
                        out_t[:].rearrange("p (c d) -> p c d", d=64)[
                            :, half * 4:(half + 1) * 4, :],
                        ps_u3[:, :, 0:64],
                        r_t[:, :, None].to_broadcast((128, 4, 64)),
                        OP.mult,
                    )
                nc.sync.dma_start(out_d[b * 128:(b + 1) * 128, :], out_t[:])

    nc.compile()
    return nc


def _get_nc(nblocks=TOK // BLK):
    if nblocks not in _CACHE:
        _CACHE[nblocks] = _build(nblocks)
    return _CACHE[nblocks]


def _prep_shared(Wq, bq, Wk, bk, Wv, bv):
    bf = ml_dtypes.bfloat16
    wq = np.ascontiguousarray(
        Wq.reshape(6, 128, 512).transpose(1, 0, 2)).astype(bf)
    wk = np.ascontiguousarray(
        Wk.reshape(6, 128, 512).transpose(1, 0, 2)).astype(bf)
    wv = np.ascontiguousarray(
        Wv.reshape(4, 128, 512).transpose(1, 0, 2)).astype(bf)
    wk7 = np.zeros((128, 512), np.float32)
    wk7[0, :] = bk
    wk7 = wk7.astype(bf)
    bvb = np.ascontiguousarray(
        np.broadcast_to(bv.reshape(8, 64), (128, 8, 64))).astype(bf)
    qb = np.ascontiguousarray(np.stack(
        [(-bq).reshape(4, 128).T, (bq + 1.0).reshape(4, 128).T],
        axis=-1)).astype(np.float32)
    return wq, wk, wv, wk7, bvb, qb


def kernel(x, guidance, Wq, bq, Wk, bk, Wv, bv):
    from concourse.bass_utils import run_bass_kernel_spmd

    x = np.asarray(x, dtype=np.float32)
    guidance = np.asarray(guidance, dtype=np.float32)
    Wq = np.asarray(Wq, dtype=np.float32)
    bq = np.asarray(bq, dtype=np.float32)
    Wk = np.asarray(Wk, dtype=np.float32)
    bk = np.asarray(bk, dtype=np.float32)
    Wv = np.asarray(Wv, dtype=np.float32)
    bv = np.asarray(bv, dtype=np.float32)

    nc = _get_nc()
    wq, wk, wv, wk7, bvb, qb = _prep_shared(Wq, bq, Wk, bk, Wv, bv)
    bf = ml_dtypes.bfloat16

    in_maps = []
    for c in range(NCORES):
        xs = np.asarray(x[c * BSH:(c + 1) * BSH]).reshape(TOK, HID)
        gs = np.asarray(guidance[c * BSH:(c + 1) * BSH]).reshape(TOK, GUID)
        xg = np.concatenate([xs, gs], axis=1)
        xgT = np.ascontiguousarray(xg.T).astype(bf)
        in_maps.append({"xgT": xgT, "wq": wq, "wk": wk, "wv": wv,
                        "wk7": wk7, "bvb": bvb, "qb": qb})

    res = run_bass_kernel_spmd(nc, in_maps, core_ids=list(range(NCORES)))
    outs = [r["out"] for r in res.results]
    return np.concatenate(outs, axis=0).reshape(B, L, H * D).astype(np.float32)
